# revision 20
# baseline (speedup 1.0000x reference)
"""Trainium2 Bass kernel for nn_AttentionPoolingTemporalEncoder.

Strategy (data-parallel over batch, 8 cores, 4 batch rows each):
  device:  h' = relu(x @ (32*Wp))          (fp8e4 DoubleRow matmuls: 2
                                            contraction rows/cycle; relu on the
                                            otherwise-idle vector engine,
                                            h' = 32*h stored bf16)
           ht = transpose(h')              (XBAR DMA transposes alternated
                                            across the sync+scalar HWDGE
                                            queues so neither serializes)
           scores' = ht @ wkq              (bf16; FWL stationary, N=8 issues
                                            pipeline at ~25ns)
           p = exp(scores'/32 + maskbias)  (no running max; scores are O(5))
           U'[h,:] = sum_s p[s,h] h'[s,:] ; Z[h] = sum_s p[s,h]
  host:    pooled = ((U'/32)/Z) @ Wv (+bv) per head; @Wo+bo; @W2+b2; LayerNorm.
"""

import sys
import threading

import numpy as np

sys.path.insert(0, "/opt/trn_rl_repo")

from contextlib import ExitStack

import concourse.tile as tile
from concourse import bacc, mybir
from concourse.bass_utils import run_bass_kernel_spmd


def _ensure_axon_ntff_hook_module():
    """Some images lack ``antenv.axon_hooks``; concourse imports it
    unconditionally when tracing is requested (e.g. via BASS_TRACE).
    Provide a minimal stand-in so that path degrades to no-trace
    instead of crashing."""
    try:
        from antenv import axon_hooks  # noqa: F401

        return
    except ImportError:
        pass
    import types

    mod = types.ModuleType("antenv.axon_hooks")
    mod._hook = None

    def set_axon_ntff_profile_hook(h):
        mod._hook = h

    def get_axon_ntff_profile_hook():
        return mod._hook

    mod.set_axon_ntff_profile_hook = set_axon_ntff_profile_hook
    mod.get_axon_ntff_profile_hook = get_axon_ntff_profile_hook
    sys.modules["antenv.axon_hooks"] = mod
    try:
        import antenv

        antenv.axon_hooks = mod
    except ImportError:
        pass


_ensure_axon_ntff_hook_module()

# Problem sizes (hardcoded per spec)
B, S, IN_DIM, E, H = 32, 4096, 1024, 512, 8
D = E // H
NCORES = 8
P = 128

# fp8 scale plan: Wp x32 so its ~N(0, 1/32) entries use the fp8e4 normal
# range; h' = 32h flows through the kernel, undone by the exp scale and a
# host-side /32 on U.
H_SCALE = 32.0
EXP_SCALE = 1.0 / H_SCALE

_nc_cache = {}
_nc_lock = threading.Lock()


def build_nc(BL=B // NCORES, S_=S, I_=IN_DIM, has_bp=False, no_mask=False, trace_label=""):
    """Build + compile the per-core Bass program.

    BL: batch rows per core. S_: sequence length. I_: input dim.
    has_bp: emit the extra K=1 matmul adding the input-projection bias.
    """
    key = (BL, S_, I_, has_bp, no_mask)
    with _nc_lock:
        if key in _nc_cache:
            return _nc_cache[key]

    IC = I_ // P        # input-dim chunks
    IC2 = IC // 2       # DoubleRow chunk pairs
    EC = E // P         # embed-dim chunks
    S_TILES = S_ // P   # sequence tiles per batch row

    f32 = mybir.dt.float32
    bf16 = mybir.dt.bfloat16
    f8 = mybir.dt.float8e4
    DR = mybir.MatmulPerfMode.DoubleRow
    EXP = mybir.ActivationFunctionType.Exp

    nc = bacc.Bacc(
        "TRN2",
        target_bir_lowering=False,
        debug=False,
        enable_asserts=False,
        num_devices=NCORES,
    )

    # DRAM I/O (per-core shapes). Main-matmul operands are fp8e4 (host-cast,
    # Wp pre-scaled x32): halves HBM traffic vs bf16 and runs the PE at
    # double rate via DoubleRow perf mode.
    xt = nc.dram_tensor("xt", [BL, IC, P, S_], f8, kind="ExternalInput").ap()
    wp = nc.dram_tensor("wp", [IC, P, E], f8, kind="ExternalInput").ap()
    wkq = nc.dram_tensor("wkq", [EC, P, H], bf16, kind="ExternalInput").ap()
    mb = nc.dram_tensor("mb", [BL, P, S_TILES], f32, kind="ExternalInput").ap()
    if has_bp:
        bp_d = nc.dram_tensor("bp", [1, E], f8, kind="ExternalInput").ap()
    u_out = nc.dram_tensor("u_out", [BL, H, E], f32, kind="ExternalOutput").ap()
    z_out = nc.dram_tensor("z_out", [BL, H, 1], f32, kind="ExternalOutput").ap()

    with tile.TileContext(nc) as tc, ExitStack() as ctx:
        const = ctx.enter_context(tc.tile_pool(name="const", bufs=1))
        xp = ctx.enter_context(tc.tile_pool(name="xp", bufs=4))
        hp = ctx.enter_context(tc.tile_pool(name="hp", bufs=7))
        # separate ht pools per HWDGE queue: buffer-reuse (WAW) chains stay
        # within one queue, avoiding cross-queue semaphore serialization
        htp_a = ctx.enter_context(tc.tile_pool(name="htp_a", bufs=4))
        htp_b = ctx.enter_context(tc.tile_pool(name="htp_b", bufs=4))
        pp = ctx.enter_context(tc.tile_pool(name="pp", bufs=4))
        mbp = ctx.enter_context(tc.tile_pool(name="mbp", bufs=BL))
        uzp = ctx.enter_context(tc.tile_pool(name="uzp", bufs=2))
        ps_h = ctx.enter_context(tc.tile_pool(name="ps_h", bufs=3, space="PSUM"))
        ps_s = ctx.enter_context(tc.tile_pool(name="ps_s", bufs=2, space="PSUM"))
        ps_u = ctx.enter_context(tc.tile_pool(name="ps_u", bufs=2, space="PSUM"))
        ps_z = ctx.enter_context(tc.tile_pool(name="ps_z", bufs=1, space="PSUM"))

        # Resident constants (wp on the scalar queue so it loads in parallel
        # with the first x chunk on sync; wkq tiny, on gpsimd)
        wp_sb = const.tile([P, IC, E], f8)
        nc.scalar.dma_start(wp_sb[:], wp.rearrange("c p e -> p c e"))
        wkq_sb = const.tile([P, EC, H], bf16)
        nc.gpsimd.dma_start(wkq_sb[:], wkq.rearrange("c p h -> p c h"))
        ones_t = const.tile([P, 2], bf16)
        nc.gpsimd.memset(ones_t[:], 1.0)
        if has_bp:
            ones_row = const.tile([1, P], f8)
            nc.gpsimd.memset(ones_row[:], 1.0)
            bp_sb = const.tile([1, E], f8)
            nc.sync.dma_start(bp_sb[:], bp_d[:])

        # Chunked x prefetch (1024 s = 8 tiles per chunk), issued ahead so
        # loads never queue behind the per-tile transposes.
        SC = min(1024, S_)
        TPC = SC // P
        NCH = S_ // SC
        chunks = [(bb, cc) for bb in range(BL) for cc in range(NCH)]

        def load_chunk(idx):
            bb, cc = chunks[idx]
            xt_c = xp.tile([P, IC, SC], f8, tag="xchunk")
            nc.sync.dma_start(
                xt_c[:],
                xt[bb, :, :, cc * SC : (cc + 1) * SC].rearrange("c p s -> p c s"),
            )
            return xt_c

        # distance-2 prefetch: two chunks in flight ahead of the consumer
        bufq = [load_chunk(0)]
        if len(chunks) > 1:
            bufq.append(load_chunk(1))
        chunk_idx = 1

        # All mask-bias tiles up front (tiny): keeps the gpsimd queue free for
        # u/z output DMAs so row boundaries don't convoy exp -> mb -> u_out.
        mb_all = []
        for b in range(BL):
            mb_t = mbp.tile([P, S_TILES], f32)
            nc.gpsimd.dma_start(mb_t[:], mb[b])
            mb_all.append(mb_t)

        for b in range(BL):
            mb_t = mb_all[b]
            u_ps = ps_u.tile([H, E], f32)
            z_ps = ps_z.tile([H, 2], f32)

            # Two-stage software-pipelined tails: tile t's scores+exp run at
            # lag 3 (transpose long done), its U/Z matmuls at lag 4 (exp long
            # done) — so the in-order tensor queue never waits at its head.
            pend_a = []  # awaiting stage A (scores+exp): (t, h_se, ht_sb)
            pend_b = []  # awaiting stage B (U+Z):       (t, h_se, p_sb)

            def stage_a(pend):
                t_, h_se_, ht_sb_ = pend
                # scores'[s,h] = sum_e h'[s,e] wkq[e,h]
                sc_ps = ps_s.tile([P, H], f32)
                for ec in range(EC):
                    nc.tensor.matmul(
                        sc_ps[:],
                        ht_sb_[:, ec, :],
                        wkq_sb[:, ec, :],
                        start=(ec == 0),
                        stop=(ec == EC - 1),
                    )
                # p = exp(scores'/32 + maskbias); maskbias = 0 for unmasked,
                # -1e4 for masked positions (additive bias port, per-partition).
                p_sb = pp.tile([P, H], bf16)
                nc.scalar.activation(
                    p_sb[:], sc_ps[:], EXP, bias=mb_t[:, t_ : t_ + 1],
                    scale=EXP_SCALE,
                )
                pend_b.append((t_, h_se_, p_sb))

            def stage_b(pend):
                t_, h_se_, p_sb_ = pend
                nc.tensor.matmul(
                    u_ps[:],
                    p_sb_[:],
                    h_se_[:],
                    start=(t_ == 0),
                    stop=(t_ == S_TILES - 1),
                    skip_group_check=True,
                )
                nc.tensor.matmul(
                    z_ps[:],
                    p_sb_[:],
                    ones_t[:],
                    start=(t_ == 0),
                    stop=(t_ == S_TILES - 1),
                    skip_group_check=True,
                )

            for t in range(S_TILES):
                    if t % TPC == 0:
                        # consume the next chunk; keep two loads in flight
                        x_sb = bufq.pop(0)
                        if chunk_idx + 1 < len(chunks):
                            chunk_idx += 1
                            bufq.append(load_chunk(chunk_idx))
                    # h' = relu(x @ 32Wp): 4 DoubleRow i-chunk-pair matmuls
                    # (each contracts 256 input rows at 2 rows/cycle)
                    h_ps = ps_h.tile([P, E], f32)
                    for c in range(IC2):
                        nc.tensor.matmul(
                            h_ps[:],
                            x_sb[:, 2 * c : 2 * c + 2, (t % TPC) * P : (t % TPC + 1) * P],
                            wp_sb[:, 2 * c : 2 * c + 2, :],
                            start=(c == 0),
                            stop=(c == IC2 - 1) and not has_bp,
                            perf_mode=DR,
                        )
                    if has_bp:
                        nc.tensor.matmul(
                            h_ps[:],
                            ones_row[:],
                            bp_sb[:],
                            start=False,
                            stop=True,
                            skip_group_check=True,
                        )
                    # relu on the (otherwise idle) vector engine
                    h_se = hp.tile([P, E], bf16)
                    nc.vector.tensor_relu(h_se[:], h_ps[:])

                    # tails first so exps never queue behind a transpose that
                    # is still waiting on its relu (in-order engine queues)
                    if len(pend_b) > 0:
                        stage_b(pend_b.pop(0))
                    if len(pend_a) > 2:
                        stage_a(pend_a.pop(0))

                    # hT via one batched DMA XBAR transpose, SBUF -> SBUF:
                    # ht_sb[e_in, ec, s] = h_se[s, ec*128 + e_in]. Alternate
                    # the two HWDGE queues so transposes don't serialize.
                    if t % 2 == 0:
                        ht_sb = htp_a.tile([P, EC, P], bf16)
                        nc.sync.dma_start_transpose(ht_sb[:], h_se[:])
                    else:
                        ht_sb = htp_b.tile([P, EC, P], bf16)
                        nc.scalar.dma_start_transpose(ht_sb[:], h_se[:])

                    pend_a.append((t, h_se, ht_sb))
            while pend_a or pend_b:
                while pend_b:
                    stage_b(pend_b.pop(0))
                if pend_a:
                    stage_a(pend_a.pop(0))

            u_sb = uzp.tile([H, E], f32, tag="u_sb")
            z_sb = uzp.tile([H, 1], f32, tag="z_sb")
            nc.vector.tensor_copy(u_sb[:], u_ps[:])
            nc.vector.tensor_copy(z_sb[:], z_ps[:, 0:1])
            nc.gpsimd.dma_start(u_out[b], u_sb[:])
            nc.gpsimd.dma_start(z_out[b], z_sb[:])

    nc.compile()
    with _nc_lock:
        _nc_cache[key] = nc
    return nc


def prepare_core_inputs(x, mask, Wp, wkq_scaled, bp=None):
    """Host-side packing for ONE core's shard.

    x: (BL, S, IN_DIM) fp32; mask: (BL, S) int; wkq_scaled: (E, H) fp32.
    """
    import ml_dtypes

    bf16 = ml_dtypes.bfloat16
    f8 = ml_dtypes.float8_e4m3
    BL_, S_, I_ = x.shape
    IC = I_ // P
    EC = E // P
    # xt[b, c, i_in, s] = x[b, s, c*128+i_in]
    xt = np.ascontiguousarray(
        x.reshape(BL_, S_, IC, P).transpose(0, 2, 3, 1)
    ).astype(f8)
    # Wp scaled x32 so its ~N(0, 1/32) entries use the fp8e4 normal range
    wp = np.ascontiguousarray(Wp.reshape(IC, P, E) * H_SCALE).astype(f8)
    wkq = np.ascontiguousarray(wkq_scaled.reshape(EC, P, H)).astype(bf16)
    # additive mask bias packed [BL, P, S_TILES]: 0 where kept, -1e4 where
    # masked (exp(-1e4 + s) underflows to exactly 0)
    mb = np.ascontiguousarray(
        ((mask.astype(np.float32) - 1.0) * 1.0e4)
        .reshape(BL_, S_ // P, P)
        .transpose(0, 2, 1)
    ).astype(np.float32)
    m = {"xt": xt, "wp": wp, "wkq": wkq, "mb": mb}
    if bp is not None:
        m["bp"] = (np.asarray(bp) * H_SCALE).astype(f8).reshape(1, E)
    return m


def kernel(
    x, mask, query, Wp, bp, Wq, bq, Wk, bk, Wv, bv, Wo, bo, W2, b2, gamma, beta,
    _trace=False,
):
    x = np.asarray(x)
    mask = np.asarray(mask)
    BL = B // NCORES

    # Host-side folds (all tiny)
    qh = (np.asarray(query, np.float64) @ np.asarray(Wq, np.float64)
          + np.asarray(bq, np.float64)).reshape(H, D)
    wkq_scaled = np.einsum(
        "ehd,hd->eh",
        np.asarray(Wk, np.float64).reshape(E, H, D),
        qh,
    ) / np.sqrt(D)

    has_bp = bool(np.any(np.asarray(bp)))
    nc = build_nc(has_bp=has_bp)

    in_maps = []
    for c in range(NCORES):
        sl = slice(c * BL, (c + 1) * BL)
        in_maps.append(
            prepare_core_inputs(
                x[sl], mask[sl], np.asarray(Wp), wkq_scaled.astype(np.float32),
                bp=np.asarray(bp) if has_bp else None,
            )
        )

    res = run_bass_kernel_spmd(
        nc, in_maps, core_ids=list(range(NCORES)), trace=_trace
    )
    U = np.concatenate([r["u_out"] for r in res.results], axis=0)  # (B, H, E)
    Z = np.concatenate([r["z_out"] for r in res.results], axis=0)[..., :1]  # (B, H, 1)

    # Host epilogue in float64 (U' = 32U from the Wp host scaling)
    pooledH = (U.astype(np.float64) / H_SCALE) / Z.astype(np.float64)  # (B, H, E)
    Wv64 = np.asarray(Wv, np.float64).reshape(E, H, D)
    pooled = np.einsum("bhe,ehd->bhd", pooledH, Wv64).reshape(B, E)
    pooled += np.asarray(bv, np.float64)
    pooled = pooled @ np.asarray(Wo, np.float64) + np.asarray(bo, np.float64)
    out = pooled @ np.asarray(W2, np.float64) + np.asarray(b2, np.float64)
    mu = out.mean(-1, keepdims=True)
    var = out.var(-1, keepdims=True)
    out = (out - mu) / np.sqrt(var + 1e-5) * np.asarray(gamma, np.float64) + np.asarray(
        beta, np.float64
    )
    out_f32 = out.astype(np.float32)
    if _trace:
        return out_f32, res
    return out_f32


# revision 21
# speedup vs baseline: 1.1715x; 1.1715x over previous
"""Trainium2 Bass kernel for nn_AttentionPoolingTemporalEncoder.

Strategy (data-parallel over batch, 8 cores, 4 batch rows each):
  device:  h' = relu(x @ (32*Wp))          (fp8e4 DoubleRow matmuls: 2
                                            contraction rows/cycle; relu on the
                                            otherwise-idle vector engine,
                                            h' = 32*h stored bf16)
           ht = transpose(h')              (XBAR DMA transposes alternated
                                            across the sync+scalar HWDGE
                                            queues so neither serializes)
           scores' = ht @ wkq              (bf16; FWL stationary, N=8 issues
                                            pipeline at ~25ns)
           p = exp(scores'/32 + maskbias)  (no running max; scores are O(5))
           U'[h,:] = sum_s p[s,h] h'[s,:] ; Z[h] = sum_s p[s,h]
  host:    pooled = ((U'/32)/Z) @ Wv (+bv) per head; @Wo+bo; @W2+b2; LayerNorm.
"""

import sys
import threading

import numpy as np

sys.path.insert(0, "/opt/trn_rl_repo")

from contextlib import ExitStack

import concourse.tile as tile
from concourse import bacc, mybir
from concourse.bass_utils import run_bass_kernel_spmd


def _ensure_axon_ntff_hook_module():
    """Some images lack ``antenv.axon_hooks``; concourse imports it
    unconditionally when tracing is requested (e.g. via BASS_TRACE).
    Provide a minimal stand-in so that path degrades to no-trace
    instead of crashing."""
    try:
        from antenv import axon_hooks  # noqa: F401

        return
    except ImportError:
        pass
    import types

    mod = types.ModuleType("antenv.axon_hooks")
    mod._hook = None

    def set_axon_ntff_profile_hook(h):
        mod._hook = h

    def get_axon_ntff_profile_hook():
        return mod._hook

    mod.set_axon_ntff_profile_hook = set_axon_ntff_profile_hook
    mod.get_axon_ntff_profile_hook = get_axon_ntff_profile_hook
    sys.modules["antenv.axon_hooks"] = mod
    try:
        import antenv

        antenv.axon_hooks = mod
    except ImportError:
        pass


_ensure_axon_ntff_hook_module()

# Problem sizes (hardcoded per spec)
B, S, IN_DIM, E, H = 32, 4096, 1024, 512, 8
D = E // H
NCORES = 8
P = 128

# fp8 scale plan: Wp x32 so its ~N(0, 1/32) entries use the fp8e4 normal
# range; h' = 32h flows through the kernel, undone by the exp scale and a
# host-side /32 on U.
H_SCALE = 32.0
EXP_SCALE = 1.0 / H_SCALE

_nc_cache = {}
_nc_lock = threading.Lock()


def build_nc(BL=B // NCORES, S_=S, I_=IN_DIM, has_bp=False, no_mask=False, trace_label=""):
    """Build + compile the per-core Bass program.

    BL: batch rows per core. S_: sequence length. I_: input dim.
    has_bp: emit the extra K=1 matmul adding the input-projection bias.
    """
    key = (BL, S_, I_, has_bp, no_mask)
    with _nc_lock:
        if key in _nc_cache:
            return _nc_cache[key]

    IC = I_ // P        # input-dim chunks
    IC2 = IC // 2       # DoubleRow chunk pairs
    EC = E // P         # embed-dim chunks
    S_TILES = S_ // P   # sequence tiles per batch row

    f32 = mybir.dt.float32
    bf16 = mybir.dt.bfloat16
    f8 = mybir.dt.float8e4
    DR = mybir.MatmulPerfMode.DoubleRow
    EXP = mybir.ActivationFunctionType.Exp

    nc = bacc.Bacc(
        "TRN2",
        target_bir_lowering=False,
        debug=False,
        enable_asserts=False,
        num_devices=NCORES,
    )

    # DRAM I/O (per-core shapes). Main-matmul operands are fp8e4 (host-cast,
    # Wp pre-scaled x32): halves HBM traffic vs bf16 and runs the PE at
    # double rate via DoubleRow perf mode.
    xt = nc.dram_tensor("xt", [BL, IC, P, S_], f8, kind="ExternalInput").ap()
    wp = nc.dram_tensor("wp", [IC, P, E], f8, kind="ExternalInput").ap()
    wkq = nc.dram_tensor("wkq", [EC, P, H], bf16, kind="ExternalInput").ap()
    mb = nc.dram_tensor("mb", [BL, P, S_TILES], f32, kind="ExternalInput").ap()
    if has_bp:
        bp_d = nc.dram_tensor("bp", [1, E], f8, kind="ExternalInput").ap()
    u_out = nc.dram_tensor("u_out", [BL, H, E], f32, kind="ExternalOutput").ap()
    z_out = nc.dram_tensor("z_out", [BL, H, 1], f32, kind="ExternalOutput").ap()

    with tile.TileContext(nc) as tc, ExitStack() as ctx:
        const = ctx.enter_context(tc.tile_pool(name="const", bufs=1))
        xp = ctx.enter_context(tc.tile_pool(name="xp", bufs=4))
        hp = ctx.enter_context(tc.tile_pool(name="hp", bufs=7))
        # separate ht pools per HWDGE queue: buffer-reuse (WAW) chains stay
        # within one queue, avoiding cross-queue semaphore serialization
        htp_a = ctx.enter_context(tc.tile_pool(name="htp_a", bufs=4))
        htp_b = ctx.enter_context(tc.tile_pool(name="htp_b", bufs=4))
        pp = ctx.enter_context(tc.tile_pool(name="pp", bufs=4))
        mbp = ctx.enter_context(tc.tile_pool(name="mbp", bufs=BL))
        uzp = ctx.enter_context(tc.tile_pool(name="uzp", bufs=2))
        ps_h = ctx.enter_context(tc.tile_pool(name="ps_h", bufs=3, space="PSUM"))
        ps_s = ctx.enter_context(tc.tile_pool(name="ps_s", bufs=2, space="PSUM"))
        ps_u = ctx.enter_context(tc.tile_pool(name="ps_u", bufs=2, space="PSUM"))
        ps_z = ctx.enter_context(tc.tile_pool(name="ps_z", bufs=1, space="PSUM"))

        # Resident constants
        wp_sb = const.tile([P, IC, E], f8)
        nc.sync.dma_start(wp_sb[:], wp.rearrange("c p e -> p c e"))
        wkq_sb = const.tile([P, EC, H], bf16)
        nc.sync.dma_start(wkq_sb[:], wkq.rearrange("c p h -> p c h"))
        ones_t = const.tile([P, 2], bf16)
        nc.gpsimd.memset(ones_t[:], 1.0)
        if has_bp:
            ones_row = const.tile([1, P], f8)
            nc.gpsimd.memset(ones_row[:], 1.0)
            bp_sb = const.tile([1, E], f8)
            nc.sync.dma_start(bp_sb[:], bp_d[:])

        # Chunked x prefetch (1024 s = 8 tiles per chunk), issued ahead so
        # loads never queue behind the per-tile transposes.
        SC = min(1024, S_)
        TPC = SC // P
        NCH = S_ // SC
        chunks = [(bb, cc) for bb in range(BL) for cc in range(NCH)]

        def load_chunk(idx):
            bb, cc = chunks[idx]
            xt_c = xp.tile([P, IC, SC], f8, tag="xchunk")
            nc.sync.dma_start(
                xt_c[:],
                xt[bb, :, :, cc * SC : (cc + 1) * SC].rearrange("c p s -> p c s"),
            )
            return xt_c

        # distance-2 prefetch: two chunks in flight ahead of the consumer
        bufq = [load_chunk(0)]
        if len(chunks) > 1:
            bufq.append(load_chunk(1))
        chunk_idx = 1

        # All mask-bias tiles up front (tiny): keeps the gpsimd queue free for
        # u/z output DMAs so row boundaries don't convoy exp -> mb -> u_out.
        mb_all = []
        for b in range(BL):
            mb_t = mbp.tile([P, S_TILES], f32)
            nc.gpsimd.dma_start(mb_t[:], mb[b])
            mb_all.append(mb_t)

        for b in range(BL):
            mb_t = mb_all[b]
            u_ps = ps_u.tile([H, E], f32)
            z_ps = ps_z.tile([H, 2], f32)

            # Two-stage software-pipelined tails: tile t's scores+exp run at
            # lag 3 (transpose long done), its U/Z matmuls at lag 4 (exp long
            # done) — so the in-order tensor queue never waits at its head.
            pend_a = []  # awaiting stage A (scores+exp): (t, h_se, ht_sb)
            pend_b = []  # awaiting stage B (U+Z):       (t, h_se, p_sb)

            def stage_a(pend):
                t_, h_se_, ht_sb_ = pend
                # scores'[s,h] = sum_e h'[s,e] wkq[e,h]
                sc_ps = ps_s.tile([P, H], f32)
                for ec in range(EC):
                    nc.tensor.matmul(
                        sc_ps[:],
                        ht_sb_[:, ec, :],
                        wkq_sb[:, ec, :],
                        start=(ec == 0),
                        stop=(ec == EC - 1),
                    )
                # p = exp(scores'/32 + maskbias); maskbias = 0 for unmasked,
                # -1e4 for masked positions (additive bias port, per-partition).
                p_sb = pp.tile([P, H], bf16)
                nc.scalar.activation(
                    p_sb[:], sc_ps[:], EXP, bias=mb_t[:, t_ : t_ + 1],
                    scale=EXP_SCALE,
                )
                pend_b.append((t_, h_se_, p_sb))

            def stage_b(pend):
                t_, h_se_, p_sb_ = pend
                nc.tensor.matmul(
                    u_ps[:],
                    p_sb_[:],
                    h_se_[:],
                    start=(t_ == 0),
                    stop=(t_ == S_TILES - 1),
                    skip_group_check=True,
                )
                nc.tensor.matmul(
                    z_ps[:],
                    p_sb_[:],
                    ones_t[:],
                    start=(t_ == 0),
                    stop=(t_ == S_TILES - 1),
                    skip_group_check=True,
                )

            for t in range(S_TILES):
                    if t % TPC == 0:
                        # consume the next chunk; keep two loads in flight
                        x_sb = bufq.pop(0)
                        if chunk_idx + 1 < len(chunks):
                            chunk_idx += 1
                            bufq.append(load_chunk(chunk_idx))
                    # h' = relu(x @ 32Wp): 4 DoubleRow i-chunk-pair matmuls
                    # (each contracts 256 input rows at 2 rows/cycle)
                    h_ps = ps_h.tile([P, E], f32)
                    for c in range(IC2):
                        nc.tensor.matmul(
                            h_ps[:],
                            x_sb[:, 2 * c : 2 * c + 2, (t % TPC) * P : (t % TPC + 1) * P],
                            wp_sb[:, 2 * c : 2 * c + 2, :],
                            start=(c == 0),
                            stop=(c == IC2 - 1) and not has_bp,
                            perf_mode=DR,
                        )
                    if has_bp:
                        nc.tensor.matmul(
                            h_ps[:],
                            ones_row[:],
                            bp_sb[:],
                            start=False,
                            stop=True,
                            skip_group_check=True,
                        )
                    # relu on the (otherwise idle) vector engine
                    h_se = hp.tile([P, E], bf16)
                    nc.vector.tensor_relu(h_se[:], h_ps[:])

                    # tails first so exps never queue behind a transpose that
                    # is still waiting on its relu (in-order engine queues)
                    if len(pend_b) > 0:
                        stage_b(pend_b.pop(0))
                    if len(pend_a) > 2:
                        stage_a(pend_a.pop(0))

                    # hT via one batched DMA XBAR transpose, SBUF -> SBUF:
                    # ht_sb[e_in, ec, s] = h_se[s, ec*128 + e_in]. Alternate
                    # the two HWDGE queues so transposes don't serialize.
                    if t % 2 == 0:
                        ht_sb = htp_a.tile([P, EC, P], bf16)
                        nc.sync.dma_start_transpose(ht_sb[:], h_se[:])
                    else:
                        ht_sb = htp_b.tile([P, EC, P], bf16)
                        nc.scalar.dma_start_transpose(ht_sb[:], h_se[:])

                    pend_a.append((t, h_se, ht_sb))
            while pend_a or pend_b:
                while pend_b:
                    stage_b(pend_b.pop(0))
                if pend_a:
                    stage_a(pend_a.pop(0))

            u_sb = uzp.tile([H, E], f32, tag="u_sb")
            z_sb = uzp.tile([H, 1], f32, tag="z_sb")
            nc.vector.tensor_copy(u_sb[:], u_ps[:])
            nc.vector.tensor_copy(z_sb[:], z_ps[:, 0:1])
            nc.gpsimd.dma_start(u_out[b], u_sb[:])
            nc.gpsimd.dma_start(z_out[b], z_sb[:])

    nc.compile()
    with _nc_lock:
        _nc_cache[key] = nc
    return nc


def prepare_core_inputs(x, mask, Wp, wkq_scaled, bp=None):
    """Host-side packing for ONE core's shard.

    x: (BL, S, IN_DIM) fp32; mask: (BL, S) int; wkq_scaled: (E, H) fp32.
    """
    import ml_dtypes

    bf16 = ml_dtypes.bfloat16
    f8 = ml_dtypes.float8_e4m3
    BL_, S_, I_ = x.shape
    IC = I_ // P
    EC = E // P
    # xt[b, c, i_in, s] = x[b, s, c*128+i_in]
    xt = np.ascontiguousarray(
        x.reshape(BL_, S_, IC, P).transpose(0, 2, 3, 1)
    ).astype(f8)
    # Wp scaled x32 so its ~N(0, 1/32) entries use the fp8e4 normal range
    wp = np.ascontiguousarray(Wp.reshape(IC, P, E) * H_SCALE).astype(f8)
    wkq = np.ascontiguousarray(wkq_scaled.reshape(EC, P, H)).astype(bf16)
    # additive mask bias packed [BL, P, S_TILES]: 0 where kept, -1e4 where
    # masked (exp(-1e4 + s) underflows to exactly 0)
    mb = np.ascontiguousarray(
        ((mask.astype(np.float32) - 1.0) * 1.0e4)
        .reshape(BL_, S_ // P, P)
        .transpose(0, 2, 1)
    ).astype(np.float32)
    m = {"xt": xt, "wp": wp, "wkq": wkq, "mb": mb}
    if bp is not None:
        m["bp"] = (np.asarray(bp) * H_SCALE).astype(f8).reshape(1, E)
    return m


def kernel(
    x, mask, query, Wp, bp, Wq, bq, Wk, bk, Wv, bv, Wo, bo, W2, b2, gamma, beta,
    _trace=False,
):
    x = np.asarray(x)
    mask = np.asarray(mask)
    BL = B // NCORES

    # Host-side folds (all tiny)
    qh = (np.asarray(query, np.float64) @ np.asarray(Wq, np.float64)
          + np.asarray(bq, np.float64)).reshape(H, D)
    wkq_scaled = np.einsum(
        "ehd,hd->eh",
        np.asarray(Wk, np.float64).reshape(E, H, D),
        qh,
    ) / np.sqrt(D)

    has_bp = bool(np.any(np.asarray(bp)))
    nc = build_nc(has_bp=has_bp)

    in_maps = []
    for c in range(NCORES):
        sl = slice(c * BL, (c + 1) * BL)
        in_maps.append(
            prepare_core_inputs(
                x[sl], mask[sl], np.asarray(Wp), wkq_scaled.astype(np.float32),
                bp=np.asarray(bp) if has_bp else None,
            )
        )

    res = run_bass_kernel_spmd(
        nc, in_maps, core_ids=list(range(NCORES)), trace=_trace
    )
    U = np.concatenate([r["u_out"] for r in res.results], axis=0)  # (B, H, E)
    Z = np.concatenate([r["z_out"] for r in res.results], axis=0)[..., :1]  # (B, H, 1)

    # Host epilogue in float64 (U' = 32U from the Wp host scaling)
    pooledH = (U.astype(np.float64) / H_SCALE) / Z.astype(np.float64)  # (B, H, E)
    Wv64 = np.asarray(Wv, np.float64).reshape(E, H, D)
    pooled = np.einsum("bhe,ehd->bhd", pooledH, Wv64).reshape(B, E)
    pooled += np.asarray(bv, np.float64)
    pooled = pooled @ np.asarray(Wo, np.float64) + np.asarray(bo, np.float64)
    out = pooled @ np.asarray(W2, np.float64) + np.asarray(b2, np.float64)
    mu = out.mean(-1, keepdims=True)
    var = out.var(-1, keepdims=True)
    out = (out - mu) / np.sqrt(var + 1e-5) * np.asarray(gamma, np.float64) + np.asarray(
        beta, np.float64
    )
    out_f32 = out.astype(np.float32)
    if _trace:
        return out_f32, res
    return out_f32
